# revision 63
# baseline (speedup 1.0000x reference)
"""Trainium2 Bass kernel for nn_Attention_XL (B=2,T=2048,C=1024,S=2048,H=16).

Sharding: (batch, head) pairs across 8 cores — core c handles batch c//4 and
heads [(c%4)*4, (c%4)*4+4). QKV projection is column-sharded by head (no
redundant FLOPs); W_proj is column-sharded; per-core partial outputs are
summed on the host (the tensor-parallel unshard step).

Per-core device program (everything stored feature-on-partition, i.e.
transposed; all matmuls fp32r):
  kcat^T = [k_xl^T (+)DMA-accumulated pos^T, kn^T]; vn computed directly in
  [t, dim] layout (one N=256 matmul chain covers all 4 heads) into v_aug,
  whose ones column later yields the softmax denominator.
  Attention per head pair, flash-style over key chunks: scores^T via
  row-tiled K=64 matmuls (two heads packed in the PE array, adjacent PSUM
  banks), one [128,1024] ACT exp per chunk (scale folded in), out^T
  accumulated in PSUM, normalization via a K=1-matmul broadcast of
  1/sumexp. Pair-0's attention is emitted before pair-1's QKV m-tiles so
  the ACT-bound loop overlaps the remaining projection work; the output
  projection is drip-fed into pair-1's l-loops; each normalize is delayed
  past the next t-chunk's first scores.
"""
import sys

sys.path.insert(0, "/opt/trn_rl_repo")

import numpy as np
import concourse.bass as bass
import concourse.bacc as bacc
import concourse.mybir as mybir
import concourse.tile as tile
from concourse.bass_utils import run_bass_kernel_spmd

F32 = mybir.dt.float32
F32R = mybir.dt.float32r
BF16 = mybir.dt.bfloat16
I16 = mybir.dt.int16
AF = mybir.ActivationFunctionType
ADD = mybir.AluOpType.add
MULT = mybir.AluOpType.mult

HD = 64          # head dim
HPC = 4          # heads per core
N_CORES = 8

# Schraudolph exp on DVE: exp(s*scale) ~= bitcast_bf16(int16(s*A + B)) with
# A = scale*2^7/ln2, B = (127-C)*2^7. Offloads part of the exp stream from
# the ACT engine (the bottleneck) to the otherwise-underutilized vector
# engine; ~3% per-element error on the offloaded chunks, which softmax
# normalization and key-averaging shrink to well under the 2e-2 budget.
# The attention*V matmuls run in bf16 (same PE rate as fp32r) so both the
# ACT path (bf16 out) and the DVE path (int16 bit-trick out) can feed them.
SCH_C = 0.0574  # centers the (1+f)*2^-f sawtooth so its mean error is ~0
DVE_EXP_MOD = 3  # l-chunks with l % MOD == REM (and l<30) run exp on DVE
DVE_EXP_REM = 1  # REM=1 keeps the DVE queue drained at t-chunk boundaries


def r(ap):
    return ap.bitcast(F32R)


def build_program(T, S, C, timing_mode=False):
    """Build + compile the per-core Bass program. Same program on all cores.

    timing_mode: big I/O tensors become Internal DRAM (no host transfer) so
    pipelined wall-clock isolates device exec; compute is unchanged."""
    L = S + T
    nL = L // 128           # key chunks
    nS = S // 128
    nT = T // 128
    nC = C // 128           # contraction chunks for qkv proj
    nTc = T // 512          # 512-wide t chunks
    R = 3 * HPC * HD        # rows of W' (768)
    scale = 1.0 / np.sqrt(HD)
    sch_a = float(scale * (2.0 ** 7) / np.log(2.0))
    sch_b = float((127.0 - SCH_C) * (2.0 ** 7))

    nc = bacc.Bacc("TRN2", target_bir_lowering=False, debug=False)

    ik = "Internal" if timing_mode else "ExternalInput"
    ok = "Internal" if timing_mode else "ExternalOutput"
    qT = nc.dram_tensor("qT", [C, T], F32, kind=ik).ap()
    wqkvT = nc.dram_tensor("wqkvT", [C, R], F32, kind=ik).ap()
    # wpP[0:64, p*C:...] = W_proj rows of head 2p; [64:128, ...] = head 2p+1
    wp4 = nc.dram_tensor("wp4", [2 * HD, 2 * C], F32, kind=ik).ap()
    kxlT = nc.dram_tensor("kxlT", [HPC * HD, S], F32, kind=ik).ap()
    posT = nc.dram_tensor("posT", [HPC * HD, S], F32, kind=ik).ap()
    vxl = nc.dram_tensor("vxl", [S, HPC * HD], BF16, kind=ik).ap()
    n_ones = max(64, 2 * (S + T) // 128)
    ones_in = nc.dram_tensor("ones", [128, n_ones], F32, kind=ik).ap()
    outT = nc.dram_tensor("outT", [C, T], F32, kind=ok).ap()
    if timing_mode:
        dummy = nc.dram_tensor("tm_in", [128, 128], F32,
                               kind="ExternalInput").ap()
        tiny = nc.dram_tensor("tm_out", [128, 128], F32,
                              kind="ExternalOutput").ap()

    with tile.TileContext(nc) as tc:
        import contextlib
        with contextlib.ExitStack() as ctx:
            persist = ctx.enter_context(tc.tile_pool(name="persist", bufs=1))
            vaugp = ctx.enter_context(tc.tile_pool(name="vaugp", bufs=1))
            att2 = ctx.enter_context(tc.tile_pool(name="att_sb", bufs=1))

            ones_sb = persist.tile([128, 64], F32, tag="ones_sb")
            qnT = [persist.tile([128, T], F32, tag=f"qnT{p}", name=f"qnT{p}")
                   for p in range(2)]
            kcatT = [persist.tile([128, L], F32, tag=f"kcatT{p}",
                                  name=f"kcatT{p}") for p in range(2)]
            y2 = [persist.tile([128, T], F32, tag=f"y2{p}", name=f"y2{p}")
                  for p in range(2)]
            v_aug4 = vaugp.tile([128, HPC * nL * 66], BF16, tag="vaug4",
                                name="v_aug4")
            va4 = v_aug4.rearrange("p (h n w) -> p h n w", h=HPC, w=66)

            # psum pools for the attention loops are entered mid-build
            # (after the big qkv pool closes); declared here for closures
            sc_ps = mm2_ps = bc_ps = pj_ps = None
            proj_group = None

            def evacuate(oA, oB):
                # free the mm2 PSUM banks ASAP at the t-chunk boundary: two
                # DVE copies to SBUF, so the next chunk's first AV matmul
                # only waits ~1.3us instead of the whole normalize chain
                oc = []
                for o in (oA, oB):
                    c = att2.tile([66, 512], F32, tag="oevac", bufs=4,
                                  name="oc")
                    nc.vector.tensor_copy(r(c[:]), o[:])
                    oc.append(c)
                return oc

            def normalize(hA, hB, oc, ts):
                # y^T = out^T * (1/sumexp), sumexp in row 64 (from the SBUF
                # evacuation copies). Even heads land on y2 partitions 0:64
                # directly; odd heads go via a staging tile + SBUF->SBUF DMA
                # partition shift to partitions 64:128 (DVE is lane-locked),
                # enabling the K=128 packed projection.
                p = hA // 2
                for h, o in ((hA, oc[0]), (hB, oc[1])):
                    bc = bc_ps.tile([64, 512], F32, tag="bc", name="bc")
                    nc.tensor.matmul(
                        bc[:], r(ones_sb[64:65, 0:64]), r(o[64:65, :]),
                        start=True, stop=True, tile_position=(64, 0))
                    brec = att2.tile([64, 512], F32, tag="brec",
                                     bufs=2, name="brec")
                    nc.vector.reciprocal(brec[:], bc[:])
                    if h % 2 == 0:
                        nc.vector.tensor_mul(
                            r(y2[p][0:64, ts]), o[0:64, :], brec[:])
                    else:
                        ystg = att2.tile([128, 512], F32, tag="ystg",
                                         bufs=2, name="ystg")
                        nc.vector.tensor_mul(
                            r(ystg[0:64, :]), o[0:64, :], brec[:])
                        # on the ACT queue: keeps the bulk-load DMA queue
                        # free of head-of-line blocking
                        nc.scalar.dma_start(r(y2[p][64:128, ts]),
                                            r(ystg[0:64, :]))

            def attention(p, proj_feed, pending, extra=None):
                hA, hB = 2 * p, 2 * p + 1
                for t in range(nTc):
                    ts = slice(t * 512, (t + 1) * 512)
                    if pending:
                        ev = evacuate(*pending[2])
                        pending = (pending[0], pending[1], ev, pending[3])
                    oA = mm2_ps.tile([66, 512], F32, tag="mm2A",
                                     name="oA")
                    oB = mm2_ps.tile([66, 512], F32, tag="mm2B", name="oB")
                    feed = proj_feed(t) if proj_feed else []
                    ets = {}

                    def av(l):
                        # AV is emitted one iteration behind the scores so
                        # the PE FIFO isn't blocked on a pending exp while
                        # the next scores (and drip matmuls) are ready
                        et = ets.pop(l)
                        nc.tensor.matmul(
                            oA[:], va4[:, hA, l, :], et[:, 0:512],
                            start=(l == 0), stop=(l == nL - 1))
                        nc.tensor.matmul(
                            oB[:], va4[:, hB, l, :], et[:, 512:1024],
                            start=(l == 0), stop=(l == nL - 1))

                    for l in range(nL):
                        if extra:
                            extra(t, l)
                        lsl = slice(l * 128, (l + 1) * 128)
                        sc = sc_ps.tile([128, 1024], F32, tag="sc", name="sc")
                        nc.tensor.matmul(
                            sc[:, 0:512],
                            r(kcatT[p][0:64, lsl]), r(qnT[p][0:64, ts]),
                            start=True, stop=True, tile_position=(0, 0))
                        nc.tensor.matmul(
                            sc[:, 512:1024],
                            r(kcatT[p][64:128, lsl]), r(qnT[p][64:128, ts]),
                            start=True, stop=True, tile_position=(64, 0))
                        et = att2.tile([128, 1024], BF16, tag="exp",
                                       bufs=2, name="et")
                        if l % DVE_EXP_MOD == DVE_EXP_REM and l < 30:
                            nc.vector.tensor_scalar(
                                et[:].bitcast(I16), sc[:], sch_a, sch_b,
                                MULT, ADD)
                        else:
                            nc.scalar.activation(et[:], sc[:], AF.Exp,
                                                 scale=float(scale))
                        ets[l] = et
                        if l == 4 and pending:
                            normalize(*pending)
                            pending = None
                        if l > 0:
                            av(l - 1)
                        if feed and l >= 8 and (l - 8) % 3 == 0:
                            d = (l - 8) // 3
                            if d < len(feed):
                                proj_group(*feed[d])
                    av(nL - 1)
                    pending = (hA, hB, (oA, oB), ts)
                return pending

            # ---- phase 1: loads + qkv projection ----
            with tc.tile_pool(name="ph1", bufs=1) as ph1:
                qTr = qT.rearrange("(n p) t -> p n t", p=128)
                wqr = wqkvT.rearrange("(n p) m -> p n m", p=128)
                # DMA order = criticality. attention t=0 l<16 touches only
                # qnT-t0, the XL kcat and the XL v_aug, so it can start ~20us
                # in; the remaining qn/kn chunks drip into its l-loop via the
                # shared vnp PSUM slot.
                # preload the Exp table set while DMAs stream
                warm = att2.tile([1, 16], F32, tag="warm", name="warm")
                nc.vector.memset(warm[0:1, 0:8], 0.0)
                nc.scalar.activation(warm[0:1, 8:16], warm[0:1, 0:8], AF.Exp)
                wqm0 = ph1.tile([128, nC, 128], F32, tag="wqm0")
                nc.sync.dma_start(r(wqm0[:]), r(wqr[:, :, 0:128]))
                wqm2 = ph1.tile([128, nC, 128], F32, tag="wqm2")
                nc.sync.dma_start(r(wqm2[:]), r(wqr[:, :, 256:384]))
                qt = ph1.tile([128, nC, T], F32, tag="qt")

                def load_qt(t):
                    ts = slice(t * 512, (t + 1) * 512)
                    for k in range(nC):
                        nc.sync.dma_start(r(qt[:, k:k + 1, ts]),
                                          r(qTr[:, k:k + 1, ts]))

                def load_kx(p, q0, q1, split=4):
                    # kcat^T kx part (pos^T DMA-accumulated), in key-blocks
                    # so attention l=0 doesn't wait for the full cache load
                    qs = S // split
                    for qq in range(q0, q1):
                        sl = slice(qq * qs, (qq + 1) * qs)
                        nc.sync.dma_start(
                            r(kcatT[p][:, sl]),
                            r(kxlT[p * 128:(p + 1) * 128, sl]))
                        nc.gpsimd.dma_start(
                            r(kcatT[p][:, sl]),
                            r(posT[p * 128:(p + 1) * 128, sl]),
                            accum_op=ADD)

                def load_v(p):
                    for h in (2 * p, 2 * p + 1):
                        nc.vector.memset(va4[:, h, :, 64:66], 1.0)
                        nc.sync.dma_start(
                            va4[:, h, 0:nS, 0:HD],
                            vxl.rearrange("(n p) d -> p n d", p=128)
                            [:, :, h * HD:(h + 1) * HD])

                def load_pair(p):
                    load_v(p)
                    load_kx(p, 0, 1, split=1)

                load_kx(0, 0, 1)
                load_v(0)
                load_qt(0)
                load_kx(0, 1, 4)
                wq45 = ph1.tile([128, nC, 2 * 128], F32, tag="wq45")
                nc.sync.dma_start(r(wq45[:]), r(wqr[:, :, 512:768]))
                for t in range(1, nTc):
                    load_qt(t)
                nc.sync.dma_start(r(ones_sb[:]), r(ones_in[:, 0:64]))
                # pair-1 loads + weights: DMA engines are otherwise idle
                # during pair-0's attention
                load_pair(1)
                wqm1 = ph1.tile([128, nC, 128], F32, tag="wqm1")
                nc.sync.dma_start(r(wqm1[:]), r(wqr[:, :, 128:256]))
                wqm3 = ph1.tile([128, nC, 128], F32, tag="wqm3")
                nc.sync.dma_start(r(wqm3[:]), r(wqr[:, :, 384:512]))

                def qkv_chunk(m, t, pool, wqm, tag="qkv"):
                    # one [128, 512] t-chunk of qkv m-tile m
                    p = m % 2
                    ts = slice(t * 512, (t + 1) * 512)
                    ps = pool.tile([128, 512], F32, tag=tag, name="ps")
                    for k in range(nC):
                        nc.tensor.matmul(
                            ps[:], r(wqm[:, k, :]), r(qt[:, k, ts]),
                            start=(k == 0), stop=(k == nC - 1))
                    dst = (qnT[p][:, ts] if m < 2 else
                           kcatT[p][:, S + t * 512:S + (t + 1) * 512])
                    nc.vector.tensor_copy(r(dst), ps[:])

                with tc.tile_pool(name="qkv1", bufs=2, space="PSUM") as qkv1:
                    qkv_chunk(0, 0, qkv1, wqm0)   # qn^T pair 0, tchunk 0
                    qkv_chunk(2, 0, qkv1, wqm2)   # kn^T pair 0, tchunk 0

                # attention psum pools (outlive ph1; LIFO within PSUM is
                # independent of the SBUF pool stack)
                sc_ps = ctx.enter_context(
                    tc.tile_pool(name="sc_ps", bufs=2, space="PSUM"))
                mm2_ps = ctx.enter_context(
                    tc.tile_pool(name="mm2_ps", bufs=1, space="PSUM"))
                bc_ps = ctx.enter_context(
                    tc.tile_pool(name="bc_ps", bufs=1, space="PSUM"))

                # vn for all 4 heads, directly in [t, dim] layout, computed
                # chunk-by-chunk inside pair-0 tchunk-0's l-loop: group j is
                # written at iteration j and first read at iteration 16+j.
                # The same PSUM slot later hosts the dripped qn/kn chunks
                # (disjoint l positions).
                with tc.tile_pool(name="vnp", bufs=1, space="PSUM") as vnp:
                    def p0_extra(t, l):
                        if t == 0:
                            # vn chunk i (read at l=16+i) starts at l=2 so
                            # the first scores don't queue behind a wq45
                            # DMA wait; i=14,15 slot in after the kn drips
                            i = (l - 2 if 2 <= l < nT else
                                 l - 3 if l in (17, 18) else None)
                            if i is not None:
                                vn = vnp.tile([128, 256], F32,
                                              tag="vn", name="vn")
                                for k in range(nC):
                                    nc.tensor.matmul(
                                        vn[:],
                                        r(qt[:, k, i * 128:(i + 1) * 128]),
                                        r(wq45[:, k, :]),
                                        start=(k == 0), stop=(k == nC - 1))
                                # one strided copy fills all heads' va rows
                                nc.vector.tensor_copy(
                                    va4[:, :, nS + i, 0:HD],
                                    vn[:].rearrange("p (h d) -> p h d",
                                                    h=HPC))
                            if l in (16, 19, 22):
                                # kn chunk t' is read from l = 16+4t' onward
                                qkv_chunk(2, 1 + (l - 16) // 3, vnp,
                                          wqm2, tag="vn")
                        if t < nTc - 1 and l == 25:
                            # qn chunk t+1 is read by the next t-loop's scores
                            qkv_chunk(0, t + 1, vnp, wqm0,
                                      tag="vn")
                        if t >= 2 and l in (3, 9, 15, 21):
                            # pair-1 qkv chunks ride pair-0's late PE slack
                            m, wqm = (1, wqm1) if t == 2 else (3, wqm3)
                            qkv_chunk(m, (l - 3) // 6, vnp, wqm,
                                      tag="vn")

                    pending = attention(0, None, None, extra=p0_extra)



            # ---- pair-1 attention with drip-fed output projection ----
            with tc.tile_pool(name="tail_sb", bufs=1) as tail, \
                 tc.tile_pool(name="pj_ps", bufs=1, space="PSUM") as pj_ps:
                wp = tail.tile([2 * HD, 2 * C], F32, tag="wp")
                nc.sync.dma_start(r(wp[:]), r(wp4[:]))

                def proj_group(t, d, at_tail=False):
                    # out^T[d-chunk, tchunk t]: one K=128 matmul per pair.
                    # Tail groups borrow the (idle) double-buffered sc pool
                    # so consecutive groups pipeline through two PSUM banks.
                    ts = slice(t * 512, (t + 1) * 512)
                    if at_tail:
                        ps = sc_ps.tile([128, 512], F32, tag="sc", name="pj")
                    else:
                        ps = pj_ps.tile([128, 512], F32, tag="proj",
                                        name="pj")
                    for p in range(2):
                        nc.tensor.matmul(
                            ps[:],
                            r(wp[:, p * C + d * 128:p * C + (d + 1) * 128]),
                            r(y2[p][:, ts]),
                            start=(p == 0), stop=(p == 1))
                    ob = tail.tile([128, 512], F32, tag="ob", bufs=4,
                                   name="ob")
                    if at_tail:
                        # ACT is idle in the epilogue; DVE would serialize
                        nc.scalar.copy(ob[:], ps[:])
                    else:
                        nc.vector.tensor_copy(ob[:], ps[:])
                    nc.sync.dma_start(outT[d * 128:(d + 1) * 128, ts], ob[:])

                def feed(t):
                    # during pair-1 tchunk t, project tchunk t-1
                    if t == 0:
                        return []
                    return [(t - 1, d) for d in range(nC)]

                pending = attention(1, feed, pending)
                ev = evacuate(*pending[2])
                normalize(pending[0], pending[1], ev, pending[3])
                for d in range(nC):
                    proj_group(nTc - 1, d, at_tail=True)
                if timing_mode:
                    tt = tail.tile([128, 128], F32, tag="tiny")
                    nc.sync.dma_start(tt[:], dummy[:])
                    nc.sync.dma_start(tiny[:], tt[:])

    nc.compile()
    return nc


_cache = {}


def _program(T, S, C):
    key = (T, S, C)
    if key not in _cache:
        _cache[key] = build_program(T, S, C)
    return _cache[key]


def core_inputs(q, k_xl, v_xl, W_qkv, W_proj, pos_emb, core):
    """Host-side shard prep for one core (slicing + layout transposes)."""
    C = q.shape[2]
    b = core // 4
    h0 = (core % 4) * HPC
    cols = slice(h0 * HD, (h0 + HPC) * HD)
    rows = np.r_[h0 * HD:(h0 + HPC) * HD]
    wrows = np.concatenate([rows, C + rows, 2 * C + rows])
    # packed pair layout: wp4[q*HD:(q+1)*HD, p*C:(p+1)*C] = head (2p+q) rows
    wp4 = (W_proj[:, cols].T.reshape(2, 2, HD, C)
           .transpose(1, 2, 0, 3).reshape(2 * HD, 2 * C))
    return {
        "qT": np.ascontiguousarray(q[b].T),
        "wqkvT": np.ascontiguousarray(W_qkv[wrows].T),
        "wp4": np.ascontiguousarray(wp4),
        "kxlT": np.ascontiguousarray(k_xl[b].T[cols]),
        "posT": np.ascontiguousarray(pos_emb.T[cols]),
        "vxl": np.ascontiguousarray(
            v_xl[b][:, cols].astype(mybir.dt.np(mybir.dt.bfloat16))),
        "ones": np.ones(
            (128, max(64, 2 * (q.shape[1] + k_xl.shape[1]) // 128)),
            np.float32),
    }


def kernel(q, k_xl, v_xl, W_qkv, W_proj, pos_emb, is_causal):
    q = np.asarray(q, np.float32)
    k_xl = np.asarray(k_xl, np.float32)
    v_xl = np.asarray(v_xl, np.float32)
    W_qkv = np.asarray(W_qkv, np.float32)
    W_proj = np.asarray(W_proj, np.float32)
    pos_emb = np.asarray(pos_emb, np.float32)
    B, T, C = q.shape
    S = k_xl.shape[1]

    nc = _program(T, S, C)
    in_maps = [core_inputs(q, k_xl, v_xl, W_qkv, W_proj, pos_emb, c)
               for c in range(N_CORES)]
    res = run_bass_kernel_spmd(nc, in_maps, list(range(N_CORES)))

    out = np.zeros((B, T, C), np.float32)
    for c in range(N_CORES):
        out[c // 4] += res.results[c]["outT"].T
    return out



# revision 65
# speedup vs baseline: 8.7492x; 8.7492x over previous
"""Trainium2 Bass kernel for nn_Attention_XL (B=2,T=2048,C=1024,S=2048,H=16).

Sharding: (batch, head) pairs across 8 cores — core c handles batch c//4 and
heads [(c%4)*4, (c%4)*4+4). QKV projection is column-sharded by head (no
redundant FLOPs); W_proj is column-sharded; per-core partial outputs are
summed on the host (the tensor-parallel unshard step).

Per-core device program (everything stored feature-on-partition, i.e.
transposed; matmuls fp32r except the bf16 attention*V):
  kcat^T = [k_xl^T (+)DMA-accumulated pos^T, kn^T]; vn computed directly in
  [t, dim] layout (one N=256 matmul chain covers all 4 heads) into v_aug
  (bf16), whose ones column later yields the softmax denominator.
  Attention per head pair, flash-style over key chunks: scores^T via
  row-tiled K=64 matmuls (two heads packed in the PE array, adjacent PSUM
  banks). The exp stream — the original bottleneck — is split between the
  ACT engine (bf16 out, scale folded in) and the DVE (Schraudolph int16
  bit-trick, see SCH_* below), sized so the PE becomes the critical engine.
  AV matmuls are emitted one iteration behind the scores (PE FIFO never
  blocks on a pending exp); accumulators are evacuated to SBUF at t-chunk
  boundaries so the next chunk's AV only waits ~1.3us. Normalization
  (K=1-matmul broadcast of sumexp, reciprocal, multiply) runs off the
  critical path; odd heads' y rows are partition-shifted via SBUF->SBUF DMA
  so the output projection contracts K=128 (both heads of a pair per
  matmul). qn/kn t-chunks beyond the first and pair-1's whole QKV are
  dripped into attention's PE slack through a shared PSUM slot, letting
  attention start ~14us in, right behind the critical DMAs (q^T tchunk-0 +
  first kcat quarter). The output projection is drip-fed into pair-1's
  l-loops; the tail 8 groups pipeline through the idle double-buffered sc
  pool with ACT doing the PSUM->SBUF copies.
"""
import sys

sys.path.insert(0, "/opt/trn_rl_repo")

import numpy as np
import concourse.bass as bass
import concourse.bacc as bacc
import concourse.mybir as mybir
import concourse.tile as tile
from concourse.bass_utils import run_bass_kernel_spmd

F32 = mybir.dt.float32
F32R = mybir.dt.float32r
BF16 = mybir.dt.bfloat16
I16 = mybir.dt.int16
AF = mybir.ActivationFunctionType
ADD = mybir.AluOpType.add
MULT = mybir.AluOpType.mult

HD = 64          # head dim
HPC = 4          # heads per core
N_CORES = 8

# Schraudolph exp on DVE: exp(s*scale) ~= bitcast_bf16(int16(s*A + B)) with
# A = scale*2^7/ln2, B = (127-C)*2^7. Offloads part of the exp stream from
# the ACT engine (the bottleneck) to the otherwise-underutilized vector
# engine; ~3% per-element error on the offloaded chunks, which softmax
# normalization and key-averaging shrink to well under the 2e-2 budget.
# The attention*V matmuls run in bf16 (same PE rate as fp32r) so both the
# ACT path (bf16 out) and the DVE path (int16 bit-trick out) can feed them.
SCH_C = 0.0574  # centers the (1+f)*2^-f sawtooth so its mean error is ~0
DVE_EXP_MOD = 3  # l-chunks with l % MOD == REM (and l<30) run exp on DVE
DVE_EXP_REM = 1  # REM=1 keeps the DVE queue drained at t-chunk boundaries


def r(ap):
    return ap.bitcast(F32R)


def build_program(T, S, C, timing_mode=False):
    """Build + compile the per-core Bass program. Same program on all cores.

    timing_mode: big I/O tensors become Internal DRAM (no host transfer) so
    pipelined wall-clock isolates device exec; compute is unchanged."""
    L = S + T
    nL = L // 128           # key chunks
    nS = S // 128
    nT = T // 128
    nC = C // 128           # contraction chunks for qkv proj
    nTc = T // 512          # 512-wide t chunks
    R = 3 * HPC * HD        # rows of W' (768)
    scale = 1.0 / np.sqrt(HD)
    sch_a = float(scale * (2.0 ** 7) / np.log(2.0))
    sch_b = float((127.0 - SCH_C) * (2.0 ** 7))

    nc = bacc.Bacc("TRN2", target_bir_lowering=False, debug=False)

    ik = "Internal" if timing_mode else "ExternalInput"
    ok = "Internal" if timing_mode else "ExternalOutput"
    qT = nc.dram_tensor("qT", [C, T], F32, kind=ik).ap()
    wqkvT = nc.dram_tensor("wqkvT", [C, R], F32, kind=ik).ap()
    # wpP[0:64, p*C:...] = W_proj rows of head 2p; [64:128, ...] = head 2p+1
    wp4 = nc.dram_tensor("wp4", [2 * HD, 2 * C], F32, kind=ik).ap()
    kxlT = nc.dram_tensor("kxlT", [HPC * HD, S], F32, kind=ik).ap()
    posT = nc.dram_tensor("posT", [HPC * HD, S], F32, kind=ik).ap()
    vxl = nc.dram_tensor("vxl", [S, HPC * HD], BF16, kind=ik).ap()
    n_ones = max(64, 2 * (S + T) // 128)
    ones_in = nc.dram_tensor("ones", [128, n_ones], F32, kind=ik).ap()
    outT = nc.dram_tensor("outT", [C, T], F32, kind=ok).ap()
    if timing_mode:
        dummy = nc.dram_tensor("tm_in", [128, 128], F32,
                               kind="ExternalInput").ap()
        tiny = nc.dram_tensor("tm_out", [128, 128], F32,
                              kind="ExternalOutput").ap()

    with tile.TileContext(nc) as tc:
        import contextlib
        with contextlib.ExitStack() as ctx:
            persist = ctx.enter_context(tc.tile_pool(name="persist", bufs=1))
            vaugp = ctx.enter_context(tc.tile_pool(name="vaugp", bufs=1))
            att2 = ctx.enter_context(tc.tile_pool(name="att_sb", bufs=1))

            ones_sb = persist.tile([128, 64], F32, tag="ones_sb")
            qnT = [persist.tile([128, T], F32, tag=f"qnT{p}", name=f"qnT{p}")
                   for p in range(2)]
            kcatT = [persist.tile([128, L], F32, tag=f"kcatT{p}",
                                  name=f"kcatT{p}") for p in range(2)]
            y2 = [persist.tile([128, T], F32, tag=f"y2{p}", name=f"y2{p}")
                  for p in range(2)]
            v_aug4 = vaugp.tile([128, HPC * nL * 66], BF16, tag="vaug4",
                                name="v_aug4")
            va4 = v_aug4.rearrange("p (h n w) -> p h n w", h=HPC, w=66)

            # psum pools for the attention loops are entered mid-build
            # (after the big qkv pool closes); declared here for closures
            sc_ps = mm2_ps = bc_ps = pj_ps = None
            proj_group = None

            def evacuate(oA, oB):
                # free the mm2 PSUM banks ASAP at the t-chunk boundary: two
                # DVE copies to SBUF, so the next chunk's first AV matmul
                # only waits ~1.3us instead of the whole normalize chain
                oc = []
                for o in (oA, oB):
                    c = att2.tile([66, 512], F32, tag="oevac", bufs=4,
                                  name="oc")
                    nc.vector.tensor_copy(r(c[:]), o[:])
                    oc.append(c)
                return oc

            def normalize(hA, hB, oc, ts):
                # y^T = out^T * (1/sumexp), sumexp in row 64 (from the SBUF
                # evacuation copies). Even heads land on y2 partitions 0:64
                # directly; odd heads go via a staging tile + SBUF->SBUF DMA
                # partition shift to partitions 64:128 (DVE is lane-locked),
                # enabling the K=128 packed projection.
                p = hA // 2
                for h, o in ((hA, oc[0]), (hB, oc[1])):
                    bc = bc_ps.tile([64, 512], F32, tag="bc", name="bc")
                    nc.tensor.matmul(
                        bc[:], r(ones_sb[64:65, 0:64]), r(o[64:65, :]),
                        start=True, stop=True, tile_position=(64, 0))
                    brec = att2.tile([64, 512], F32, tag="brec",
                                     bufs=2, name="brec")
                    nc.vector.reciprocal(brec[:], bc[:])
                    if h % 2 == 0:
                        nc.vector.tensor_mul(
                            r(y2[p][0:64, ts]), o[0:64, :], brec[:])
                    else:
                        ystg = att2.tile([128, 512], F32, tag="ystg",
                                         bufs=2, name="ystg")
                        nc.vector.tensor_mul(
                            r(ystg[0:64, :]), o[0:64, :], brec[:])
                        # on the ACT queue: keeps the bulk-load DMA queue
                        # free of head-of-line blocking
                        nc.scalar.dma_start(r(y2[p][64:128, ts]),
                                            r(ystg[0:64, :]))

            def attention(p, proj_feed, pending, extra=None):
                hA, hB = 2 * p, 2 * p + 1
                for t in range(nTc):
                    ts = slice(t * 512, (t + 1) * 512)
                    if pending:
                        ev = evacuate(*pending[2])
                        pending = (pending[0], pending[1], ev, pending[3])
                    oA = mm2_ps.tile([66, 512], F32, tag="mm2A",
                                     name="oA")
                    oB = mm2_ps.tile([66, 512], F32, tag="mm2B", name="oB")
                    feed = proj_feed(t) if proj_feed else []
                    ets = {}

                    def av(l):
                        # AV is emitted one iteration behind the scores so
                        # the PE FIFO isn't blocked on a pending exp while
                        # the next scores (and drip matmuls) are ready
                        et = ets.pop(l)
                        nc.tensor.matmul(
                            oA[:], va4[:, hA, l, :], et[:, 0:512],
                            start=(l == 0), stop=(l == nL - 1))
                        nc.tensor.matmul(
                            oB[:], va4[:, hB, l, :], et[:, 512:1024],
                            start=(l == 0), stop=(l == nL - 1))

                    for l in range(nL):
                        if extra:
                            extra(t, l)
                        lsl = slice(l * 128, (l + 1) * 128)
                        sc = sc_ps.tile([128, 1024], F32, tag="sc", name="sc")
                        nc.tensor.matmul(
                            sc[:, 0:512],
                            r(kcatT[p][0:64, lsl]), r(qnT[p][0:64, ts]),
                            start=True, stop=True, tile_position=(0, 0))
                        nc.tensor.matmul(
                            sc[:, 512:1024],
                            r(kcatT[p][64:128, lsl]), r(qnT[p][64:128, ts]),
                            start=True, stop=True, tile_position=(64, 0))
                        et = att2.tile([128, 1024], BF16, tag="exp",
                                       bufs=2, name="et")
                        if l % DVE_EXP_MOD == DVE_EXP_REM and l < 30:
                            nc.vector.tensor_scalar(
                                et[:].bitcast(I16), sc[:], sch_a, sch_b,
                                MULT, ADD)
                        else:
                            nc.scalar.activation(et[:], sc[:], AF.Exp,
                                                 scale=float(scale))
                        ets[l] = et
                        if l == 4 and pending:
                            normalize(*pending)
                            pending = None
                        if l > 0:
                            av(l - 1)
                        if feed and l >= 8 and (l - 8) % 3 == 0:
                            d = (l - 8) // 3
                            if d < len(feed):
                                proj_group(*feed[d])
                    av(nL - 1)
                    pending = (hA, hB, (oA, oB), ts)
                return pending

            # ---- phase 1: loads + qkv projection ----
            with tc.tile_pool(name="ph1", bufs=1) as ph1:
                qTr = qT.rearrange("(n p) t -> p n t", p=128)
                wqr = wqkvT.rearrange("(n p) m -> p n m", p=128)
                # DMA order = criticality. attention t=0 l<16 touches only
                # qnT-t0, the XL kcat and the XL v_aug, so it can start ~20us
                # in; the remaining qn/kn chunks drip into its l-loop via the
                # shared vnp PSUM slot.
                # preload the Exp table set while DMAs stream
                warm = att2.tile([1, 16], F32, tag="warm", name="warm")
                nc.vector.memset(warm[0:1, 0:8], 0.0)
                nc.scalar.activation(warm[0:1, 8:16], warm[0:1, 0:8], AF.Exp)
                wqm0 = ph1.tile([128, nC, 128], F32, tag="wqm0")
                nc.sync.dma_start(r(wqm0[:]), r(wqr[:, :, 0:128]))
                wqm2 = ph1.tile([128, nC, 128], F32, tag="wqm2")
                nc.sync.dma_start(r(wqm2[:]), r(wqr[:, :, 256:384]))
                qt = ph1.tile([128, nC, T], F32, tag="qt")

                def load_qt(t):
                    ts = slice(t * 512, (t + 1) * 512)
                    for k in range(nC):
                        nc.sync.dma_start(r(qt[:, k:k + 1, ts]),
                                          r(qTr[:, k:k + 1, ts]))

                def load_kx(p, q0, q1, split=4):
                    # kcat^T kx part (pos^T DMA-accumulated), in key-blocks
                    # so attention l=0 doesn't wait for the full cache load
                    qs = S // split
                    for qq in range(q0, q1):
                        sl = slice(qq * qs, (qq + 1) * qs)
                        nc.sync.dma_start(
                            r(kcatT[p][:, sl]),
                            r(kxlT[p * 128:(p + 1) * 128, sl]))
                        nc.gpsimd.dma_start(
                            r(kcatT[p][:, sl]),
                            r(posT[p * 128:(p + 1) * 128, sl]),
                            accum_op=ADD)

                def load_v(p):
                    for h in (2 * p, 2 * p + 1):
                        nc.vector.memset(va4[:, h, :, 64:66], 1.0)
                        nc.sync.dma_start(
                            va4[:, h, 0:nS, 0:HD],
                            vxl.rearrange("(n p) d -> p n d", p=128)
                            [:, :, h * HD:(h + 1) * HD])

                def load_pair(p):
                    load_v(p)
                    load_kx(p, 0, 1, split=1)

                load_kx(0, 0, 1)
                load_v(0)
                load_qt(0)
                load_kx(0, 1, 4)
                wq45 = ph1.tile([128, nC, 2 * 128], F32, tag="wq45")
                nc.sync.dma_start(r(wq45[:]), r(wqr[:, :, 512:768]))
                for t in range(1, nTc):
                    load_qt(t)
                nc.sync.dma_start(r(ones_sb[:]), r(ones_in[:, 0:64]))
                # pair-1 loads + weights: DMA engines are otherwise idle
                # during pair-0's attention
                load_pair(1)
                wqm1 = ph1.tile([128, nC, 128], F32, tag="wqm1")
                nc.sync.dma_start(r(wqm1[:]), r(wqr[:, :, 128:256]))
                wqm3 = ph1.tile([128, nC, 128], F32, tag="wqm3")
                nc.sync.dma_start(r(wqm3[:]), r(wqr[:, :, 384:512]))

                def qkv_chunk(m, t, pool, wqm, tag="qkv"):
                    # one [128, 512] t-chunk of qkv m-tile m
                    p = m % 2
                    ts = slice(t * 512, (t + 1) * 512)
                    ps = pool.tile([128, 512], F32, tag=tag, name="ps")
                    for k in range(nC):
                        nc.tensor.matmul(
                            ps[:], r(wqm[:, k, :]), r(qt[:, k, ts]),
                            start=(k == 0), stop=(k == nC - 1))
                    dst = (qnT[p][:, ts] if m < 2 else
                           kcatT[p][:, S + t * 512:S + (t + 1) * 512])
                    nc.vector.tensor_copy(r(dst), ps[:])

                with tc.tile_pool(name="qkv1", bufs=2, space="PSUM") as qkv1:
                    qkv_chunk(0, 0, qkv1, wqm0)   # qn^T pair 0, tchunk 0
                    qkv_chunk(2, 0, qkv1, wqm2)   # kn^T pair 0, tchunk 0

                # attention psum pools (outlive ph1; LIFO within PSUM is
                # independent of the SBUF pool stack)
                sc_ps = ctx.enter_context(
                    tc.tile_pool(name="sc_ps", bufs=2, space="PSUM"))
                mm2_ps = ctx.enter_context(
                    tc.tile_pool(name="mm2_ps", bufs=1, space="PSUM"))
                bc_ps = ctx.enter_context(
                    tc.tile_pool(name="bc_ps", bufs=1, space="PSUM"))

                # vn for all 4 heads, directly in [t, dim] layout, computed
                # chunk-by-chunk inside pair-0 tchunk-0's l-loop: group j is
                # written at iteration j and first read at iteration 16+j.
                # The same PSUM slot later hosts the dripped qn/kn chunks
                # (disjoint l positions).
                with tc.tile_pool(name="vnp", bufs=1, space="PSUM") as vnp:
                    def p0_extra(t, l):
                        if t == 0:
                            # vn chunk i (read at l=16+i) starts at l=2 so
                            # the first scores don't queue behind a wq45
                            # DMA wait; i=14,15 slot in after the kn drips
                            i = (l - 2 if 2 <= l < nT else
                                 l - 3 if l in (17, 18) else None)
                            if i is not None:
                                vn = vnp.tile([128, 256], F32,
                                              tag="vn", name="vn")
                                for k in range(nC):
                                    nc.tensor.matmul(
                                        vn[:],
                                        r(qt[:, k, i * 128:(i + 1) * 128]),
                                        r(wq45[:, k, :]),
                                        start=(k == 0), stop=(k == nC - 1))
                                # one strided copy fills all heads' va rows
                                nc.vector.tensor_copy(
                                    va4[:, :, nS + i, 0:HD],
                                    vn[:].rearrange("p (h d) -> p h d",
                                                    h=HPC))
                            if l in (16, 19, 22):
                                # kn chunk t' is read from l = 16+4t' onward
                                qkv_chunk(2, 1 + (l - 16) // 3, vnp,
                                          wqm2, tag="vn")
                        if t < nTc - 1 and l == 25:
                            # qn chunk t+1 is read by the next t-loop's scores
                            qkv_chunk(0, t + 1, vnp, wqm0,
                                      tag="vn")
                        if t >= 2 and l in (3, 9, 15, 21):
                            # pair-1 qkv chunks ride pair-0's late PE slack
                            m, wqm = (1, wqm1) if t == 2 else (3, wqm3)
                            qkv_chunk(m, (l - 3) // 6, vnp, wqm,
                                      tag="vn")

                    pending = attention(0, None, None, extra=p0_extra)


            # ---- pair-1 attention with drip-fed output projection ----
            with tc.tile_pool(name="tail_sb", bufs=1) as tail, \
                 tc.tile_pool(name="pj_ps", bufs=1, space="PSUM") as pj_ps:
                wp = tail.tile([2 * HD, 2 * C], F32, tag="wp")
                nc.sync.dma_start(r(wp[:]), r(wp4[:]))

                def proj_group(t, d, at_tail=False):
                    # out^T[d-chunk, tchunk t]: one K=128 matmul per pair.
                    # Tail groups borrow the (idle) double-buffered sc pool
                    # so consecutive groups pipeline through two PSUM banks.
                    ts = slice(t * 512, (t + 1) * 512)
                    if at_tail:
                        ps = sc_ps.tile([128, 512], F32, tag="sc", name="pj")
                    else:
                        ps = pj_ps.tile([128, 512], F32, tag="proj",
                                        name="pj")
                    for p in range(2):
                        nc.tensor.matmul(
                            ps[:],
                            r(wp[:, p * C + d * 128:p * C + (d + 1) * 128]),
                            r(y2[p][:, ts]),
                            start=(p == 0), stop=(p == 1))
                    ob = tail.tile([128, 512], F32, tag="ob", bufs=4,
                                   name="ob")
                    if at_tail:
                        # ACT is idle in the epilogue; DVE would serialize
                        nc.scalar.copy(ob[:], ps[:])
                    else:
                        nc.vector.tensor_copy(ob[:], ps[:])
                    nc.sync.dma_start(outT[d * 128:(d + 1) * 128, ts], ob[:])

                def feed(t):
                    # during pair-1 tchunk t, project tchunk t-1
                    if t == 0:
                        return []
                    return [(t - 1, d) for d in range(nC)]

                pending = attention(1, feed, pending)
                ev = evacuate(*pending[2])
                normalize(pending[0], pending[1], ev, pending[3])
                for d in range(nC):
                    proj_group(nTc - 1, d, at_tail=True)
                if timing_mode:
                    tt = tail.tile([128, 128], F32, tag="tiny")
                    nc.sync.dma_start(tt[:], dummy[:])
                    nc.sync.dma_start(tiny[:], tt[:])

    nc.compile()
    return nc


_cache = {}


def _program(T, S, C):
    key = (T, S, C)
    if key not in _cache:
        _cache[key] = build_program(T, S, C)
    return _cache[key]


def core_inputs(q, k_xl, v_xl, W_qkv, W_proj, pos_emb, core):
    """Host-side shard prep for one core (slicing + layout transposes)."""
    C = q.shape[2]
    b = core // 4
    h0 = (core % 4) * HPC
    cols = slice(h0 * HD, (h0 + HPC) * HD)
    rows = np.r_[h0 * HD:(h0 + HPC) * HD]
    wrows = np.concatenate([rows, C + rows, 2 * C + rows])
    # packed pair layout: wp4[q*HD:(q+1)*HD, p*C:(p+1)*C] = head (2p+q) rows
    wp4 = (W_proj[:, cols].T.reshape(2, 2, HD, C)
           .transpose(1, 2, 0, 3).reshape(2 * HD, 2 * C))
    return {
        "qT": np.ascontiguousarray(q[b].T),
        "wqkvT": np.ascontiguousarray(W_qkv[wrows].T),
        "wp4": np.ascontiguousarray(wp4),
        "kxlT": np.ascontiguousarray(k_xl[b].T[cols]),
        "posT": np.ascontiguousarray(pos_emb.T[cols]),
        "vxl": np.ascontiguousarray(
            v_xl[b][:, cols].astype(mybir.dt.np(mybir.dt.bfloat16))),
        "ones": np.ones(
            (128, max(64, 2 * (q.shape[1] + k_xl.shape[1]) // 128)),
            np.float32),
    }


def kernel(q, k_xl, v_xl, W_qkv, W_proj, pos_emb, is_causal):
    q = np.asarray(q, np.float32)
    k_xl = np.asarray(k_xl, np.float32)
    v_xl = np.asarray(v_xl, np.float32)
    W_qkv = np.asarray(W_qkv, np.float32)
    W_proj = np.asarray(W_proj, np.float32)
    pos_emb = np.asarray(pos_emb, np.float32)
    B, T, C = q.shape
    S = k_xl.shape[1]

    nc = _program(T, S, C)
    in_maps = [core_inputs(q, k_xl, v_xl, W_qkv, W_proj, pos_emb, c)
               for c in range(N_CORES)]
    res = run_bass_kernel_spmd(nc, in_maps, list(range(N_CORES)))

    out = np.zeros((B, T, C), np.float32)
    for c in range(N_CORES):
        out[c // 4] += res.results[c]["outT"].T
    return out

